# revision 3
# baseline (speedup 1.0000x reference)
"""Trainium2 Bass kernel for nn_AttentionBlock (dense_cnn).

Computes, per batch b:
    a = sigmoid(MLP(x))              # per-pixel 2048->64->16->8->1 w/ ReLU
    out[b] = sum_p(a*x) / sum_p(a)   # weighted GAP over 14x14 pixels

Sharding: pure data parallelism over batch (B=64) across 8 NeuronCores
(8 batches/core); weights replicated; no cross-core communication.

Per-core strategy (v4):
  - Bytes: chain copy of x (channel-on-partition, host-pretransposed)
    + W1 ship fp8e4m3; GAP copy ships fp16 (fp8 fails the 2e-2 gate).
    Padding eliminated: 1568 real pixels = 12 full 128-tiles + one
    32-pixel tile; ~9.65 MB bulk.  One ring saturates the ~435 GB/s
    fabric, so multi-queue splits don't help; bytes are the floor.
  - All bulk rides the scalar HWDGE ring: xT supers first (xT0 split
    in halves so h1 starts at first-half arrival), then xg in groups
    (3,3,2,2,1,1 tiles) with the tiny tile-12 block last so the GAP
    tail after the final byte is one tile + finalize.  Consts + W1
    ride the sync ring.  Trigger instructions cost ~0.6us of engine
    time each: the first six are front-loaded, the rest interleave
    between ACT ops -- each lands >5us before the ring reaches its
    transfer, so the ring never starves (v3 lost ~3us to this).
  - h1 runs fp8 DoubleRow (2 channels/PE-cell, host packs W1 and xT
    chunk-major so pairs line up): 8 matmuls per 512-pixel super at
    ~215ns vs 16, saving ~5us of PE.  ReLU+bias fuses into the ACT
    PSUM->SBUF copy; h2/h3 use one-op DVE add+max; the a-matmuls per
    super batch into one PSUM tile with a single sigmoid.
  - GAP runs per-TILE (stationary A = a*mask, one PSUM bank per
    512-chan quarter, all tiles accumulating into a 32-partition
    strip) emitted after the chain so each tile's matmuls fire as its
    xg lands, hugging the DMA stream.
  - Finalize: four scaled [8,512] PSUM->SBUF copies alternating
    ACT/DVE chained to tile-12's per-bank matmuls, two [8,1024] out
    DMAs on the idle sync ring.
  Measured: v2.8 53.4us -> v3 45.5us -> v4 target ~40us.  Fixed costs
  (per trace): ~6us preamble excluded from exec, ~8.7us semaphore-
  clear teardown included, ~1.9us trigger->first-byte, ~1.2us DMA
  ramp-up, rel err 5.5e-3.
"""

import numpy as np
import ml_dtypes
from contextlib import ExitStack

from concourse import bacc, mybir, tile
from concourse.bass_utils import run_bass_kernel_spmd

F32 = mybir.dt.float32
F16 = mybir.dt.float16
FP8 = mybir.dt.float8e4
AF = mybir.ActivationFunctionType
DR = mybir.MatmulPerfMode.DoubleRow

NP_F16 = np.float16
NP_FP8 = ml_dtypes.float8_e4m3

B, HH, WW, C = 64, 14, 14, 2048
NCORES = 8
BPC = B // NCORES            # 8 batches per core
PIX = HH * WW                # 196 pixels per batch
NPIX = BPC * PIX             # 1568 real pixels per core (no padding)
P = 128
NCH = C // P                 # 16 channel chunks
D1, D2, D3 = 64, 16, 8

NT = 13                      # 12 full 128-pixel tiles + one 32-pixel tile
TILE_SZ = [P] * 12 + [32]
TILE_OFF = [t * P for t in range(NT)]
SUPER = [[0, 1, 2, 3], [4, 5, 6, 7], [8, 9, 10, 11], [12]]
SUP_SZ = [512, 512, 512, 32]
XT_COLS = NCH * NPIX         # 25088

# xg transfer groups over tiles 0-11 (tile 12 ships separately, last)
XG_GROUPS = [[0, 1, 2], [3, 4, 5], [6, 7], [8, 9], [10], [11]]

# fp16 const blob layout (columns)
MASK_OFF = 0                   # [128, NT*32]
ONES_OFF = MASK_OFF + NT * 32  # [128, 2]
W2_OFF = ONES_OFF + 2          # [64, 16]
W3_OFF = W2_OFF + D2           # [16, 8]
W4_OFF = W3_OFF + D3           # [8, 2]
CF16_COLS = W4_OFF + 2


def build_program(b4_val: float):
    nc = bacc.Bacc("TRN2", target_bir_lowering=False, debug=False)

    xt_d = nc.dram_tensor("xt", [P, XT_COLS], FP8, kind="ExternalInput")
    w18_d = nc.dram_tensor("w18", [P, NCH * D1], FP8, kind="ExternalInput")
    xg_d = nc.dram_tensor("xg", [P, 12 * C], F16, kind="ExternalInput")
    xg3_d = nc.dram_tensor("xg3", [32, C], F16, kind="ExternalInput")
    cf16_d = nc.dram_tensor("cf16", [P, CF16_COLS], F16, kind="ExternalInput")
    bia_d = nc.dram_tensor("bia", [D1, 3], F32, kind="ExternalInput")
    out_d = nc.dram_tensor("out", [BPC, C], F32, kind="ExternalOutput")

    with tile.TileContext(nc) as tc, ExitStack() as ctx:
        const = ctx.enter_context(tc.tile_pool(name="const", bufs=1))
        acc = ctx.enter_context(tc.tile_pool(name="acc", bufs=1))
        xgp = ctx.enter_context(tc.tile_pool(name="xg", bufs=1))
        xtp = ctx.enter_context(tc.tile_pool(name="xT", bufs=len(SUPER)))
        hpool = ctx.enter_context(tc.tile_pool(name="hsb", bufs=3))
        misc = ctx.enter_context(tc.tile_pool(name="misc", bufs=14))
        ps_chain = ctx.enter_context(tc.tile_pool(name="chain", bufs=2, space="PSUM"))
        ps_h1 = ctx.enter_context(tc.tile_pool(name="h1ps", bufs=2, space="PSUM"))
        ps_gap = ctx.enter_context(tc.tile_pool(name="gap", bufs=1, space="PSUM"))

        # ---- consts + W1 on the sync ring (small; frees the scalar
        # ring to start on chain data immediately) ----
        w18 = const.tile([P, NCH, D1], FP8)
        nc.sync.dma_start(w18[:].rearrange("p k d -> p (k d)"), w18_d[:])
        cf16 = const.tile([P, CF16_COLS], F16)
        nc.sync.dma_start(cf16[:], cf16_d[:])
        bia = const.tile([D1, 3], F32)
        nc.sync.dma_start(bia[:], bia_d[:])

        # ---- bulk on the scalar HWDGE ring, chain data first ----
        xts = [None] * len(SUPER)
        for si, s_sz in enumerate(SUP_SZ):
            c0 = NCH * TILE_OFF[SUPER[si][0]]
            xT = xtp.tile([P, NCH, s_sz], FP8, tag="xT")
            if si == 0:
                # split in halves so h1(0) starts after ~half the bytes
                half = NCH // 2 * s_sz
                nc.scalar.dma_start(
                    xT[:, 0:NCH // 2, :].rearrange("p k s -> p (k s)"),
                    xt_d[:, c0:c0 + half])
                nc.scalar.dma_start(
                    xT[:, NCH // 2:NCH, :].rearrange("p k s -> p (k s)"),
                    xt_d[:, c0 + half:c0 + 2 * half])
            else:
                nc.scalar.dma_start(xT[:].rearrange("p k s -> p (k s)"),
                                    xt_d[:, c0:c0 + NCH * s_sz])
            xts[si] = xT

        xg_tiles = {}

        def emit_xg_group(gi):
            tl = XG_GROUPS[gi]
            xg = xgp.tile([P, len(tl) * C], F16, tag=f"xg{gi}", bufs=1)
            nc.scalar.dma_start(xg[:], xg_d[:, tl[0] * C:(tl[-1] + 1) * C])
            for i, t in enumerate(tl):
                xg_tiles[t] = xg[:, i * C:(i + 1) * C]

        def emit_xg12():
            xg = xgp.tile([32, C], F16, tag="xgt12", bufs=1)
            nc.scalar.dma_start(xg[:], xg3_d[:])
            xg_tiles[12] = xg[:]

        # first xg group up front; the rest interleave with ACT work
        emit_xg_group(0)

        def w1v(g):
            return w18[:, 2 * g:2 * g + 2, :]

        def maskv(t):
            return cf16[:, MASK_OFF + t * 32:MASK_OFF + (t + 1) * 32]

        onesv = cf16[:, ONES_OFF:ONES_OFF + 2]
        w2v = cf16[0:D1, W2_OFF:W2_OFF + D2]
        w3v = cf16[0:D2, W3_OFF:W3_OFF + D3]
        w4v = cf16[0:D3, W4_OFF:W4_OFF + 2]
        b1v = bia[0:D1, 0:1]
        b2v = bia[0:D2, 1:2]
        b3v = bia[0:D3, 2:3]

        cnt_sb = acc.tile([BPC, 1], F32)
        nc.vector.memset(cnt_sb[:], 0.0)

        # single-strip GAP accumulator: all tiles accumulate into
        # partitions 0-31 (batches in rows 0-7), one bank per quarter
        gap_ps = ps_gap.tile([P, 4, 512], F32)

        As = {}
        state = {}

        def h1_front(si):
            s_sz = SUP_SZ[si]
            h1_ps = ps_h1.tile([D1, s_sz], F32, tag="h1ps")
            if si < 3:
                for g in range(4):
                    nc.tensor.matmul(h1_ps[:], w1v(g), xts[si][:, 2 * g:2 * g + 2, :],
                                     start=(g == 0), stop=False, perf_mode=DR)
            else:
                # tiny super: FD=32 < 128 makes DoubleRow a net loss
                for k in range(NCH // 2):
                    nc.tensor.matmul(h1_ps[:], w18[:, k, :], xts[si][:, k, :],
                                     start=(k == 0), stop=False)
            state[si] = h1_ps

        def h1_back(si):
            h1_ps = state[si]
            if si < 3:
                for g in range(4, 8):
                    nc.tensor.matmul(h1_ps[:], w1v(g), xts[si][:, 2 * g:2 * g + 2, :],
                                     start=False, stop=(g == 7), perf_mode=DR)
            else:
                for k in range(NCH // 2, NCH):
                    nc.tensor.matmul(h1_ps[:], w18[:, k, :], xts[si][:, k, :],
                                     start=False, stop=(k == NCH - 1))

        def tail_a(si):
            s_sz = SUP_SZ[si]
            h1_sb = hpool.tile([D1, s_sz], F16, tag="h1")
            nc.scalar.activation(h1_sb[:], state[si][:], AF.Relu, bias=b1v)
            h2_ps = ps_chain.tile([D2, s_sz], F32, tag="chain")
            nc.tensor.matmul(h2_ps[:], w2v, h1_sb[:], start=True, stop=True)
            state[si] = h2_ps

        def tail_b(si):
            # relu+bias in one DVE op keeps ACT free
            s_sz = SUP_SZ[si]
            h2_sb = hpool.tile([D2, s_sz], F16, tag="h2")
            nc.vector.tensor_scalar(h2_sb[:], state[si][:], b2v, 0.0,
                                    mybir.AluOpType.add, mybir.AluOpType.max)
            h3_ps = ps_chain.tile([D3, s_sz], F32, tag="chain")
            nc.tensor.matmul(h3_ps[:], w3v, h2_sb[:], start=True, stop=True)
            state[si] = h3_ps

        def tail_c(si):
            tlist = SUPER[si]
            nts = len(tlist)
            s_sz = SUP_SZ[si]
            h3_sb = hpool.tile([D3, s_sz], F16, tag="h3")
            nc.vector.tensor_scalar(h3_sb[:], state[si][:], b3v, 0.0,
                                    mybir.AluOpType.add, mybir.AluOpType.max)
            # one PSUM tile + one sigmoid for the whole super
            a_ps4 = ps_chain.tile([s_sz // nts, 2 * nts], F32, tag="chain")
            for i in range(nts):
                tsz = TILE_SZ[tlist[i]]
                nc.tensor.matmul(a_ps4[0:tsz, 2 * i:2 * i + 2],
                                 h3_sb[:, i * P:i * P + tsz], w4v,
                                 start=True, stop=True, skip_group_check=True)
            a4 = misc.tile([s_sz // nts, 2 * nts], F16, tag="a")
            nc.scalar.activation(a4[:], a_ps4[:], AF.Sigmoid, bias=b4_val)
            cnt_ps = ps_chain.tile([32, 2], F32, tag="chain")
            for i, t in enumerate(tlist):
                tsz = TILE_SZ[t]
                A = misc.tile([tsz, 32], F16, tag="A")
                nc.vector.tensor_mul(
                    A[:], a4[0:tsz, 2 * i:2 * i + 1].to_broadcast([tsz, 32]),
                    maskv(t)[0:tsz, :])
                As[t] = A
                nc.tensor.matmul(cnt_ps[:], A[:], onesv[0:tsz, :],
                                 start=(i == 0), stop=(i == nts - 1))
            nc.vector.tensor_add(cnt_sb[:], cnt_sb[:], cnt_ps[0:BPC, 0:1])

        def gap_tile(t):
            for n in range(4):
                nc.tensor.matmul(
                    gap_ps[0:32, n, :], As[t][:],
                    xg_tiles[t][:, n * 512:(n + 1) * 512],
                    start=(t == 0), stop=(t == NT - 1),
                    tile_position=(0, 0), skip_group_check=True,
                )

        # ---- software pipeline: h1(s+1) halves interleave super s's
        # tail; remaining xg triggers slot between ACT ops (each lands
        # >5us before the ring reaches that transfer) ----
        h1_front(0)
        h1_back(0)

        h1_front(1)
        tail_a(0)
        h1_back(1)
        tail_b(0)
        emit_xg_group(1)
        emit_xg_group(2)
        h1_front(2)
        tail_a(1)
        h1_back(2)
        tail_b(1)
        emit_xg_group(3)
        emit_xg_group(4)
        tail_c(0)
        h1_front(3)
        tail_a(2)
        h1_back(3)
        tail_b(2)
        emit_xg_group(5)
        emit_xg12()
        tail_c(1)
        tail_a(3)
        tail_b(3)
        tail_c(2)
        tail_c(3)

        recip = acc.tile([BPC, 1], F32)
        nc.vector.reciprocal(recip[:], cnt_sb[:])

        # ---- GAP: per-tile, hugging the xg DMA stream; tile 12 last
        # (its 0.13 MB transfer is the final ring entry) ----
        for t in range(NT):
            gap_tile(t)

        # ---- finalize: four scaled [8,512] copies alternating
        # ACT/DVE, two [8,1024] out DMAs on the idle sync ring ----
        out_sb = acc.tile([BPC, C], F32)
        for n in range(4):
            if n % 2 == 0:
                nc.scalar.activation(out_sb[:, n * 512:(n + 1) * 512],
                                     gap_ps[0:BPC, n, :], AF.Copy,
                                     scale=recip[:])
            else:
                nc.vector.tensor_scalar_mul(out_sb[:, n * 512:(n + 1) * 512],
                                            gap_ps[0:BPC, n, :], recip[:])
            if n == 1:
                nc.sync.dma_start(out_d[:, 0:1024], out_sb[:, 0:1024])
        nc.sync.dma_start(out_d[:, 1024:2048], out_sb[:, 1024:2048])

    nc.compile()
    return nc


def _make_mask():
    m = np.zeros((P, NT * 32), dtype=np.float32)
    for t in range(NT):
        for p in range(TILE_SZ[t]):
            gp = t * P + p
            m[p, t * 32 + gp // PIX] = 1.0
    return m


def make_in_maps(x, W1, b1, W2, b2, W3, b3, W4, b4):
    x = np.ascontiguousarray(np.asarray(x, dtype=np.float32))
    cf16 = np.zeros((P, CF16_COLS), dtype=NP_F16)
    w18 = np.ascontiguousarray(
        np.asarray(W1, np.float32).reshape(NCH, P, D1).transpose(1, 0, 2)
        .reshape(P, NCH * D1)).astype(NP_FP8)
    cf16[:, MASK_OFF:MASK_OFF + NT * 32] = _make_mask().astype(NP_F16)
    cf16[:, ONES_OFF:ONES_OFF + 2] = 1.0
    cf16[0:D1, W2_OFF:W2_OFF + D2] = np.asarray(W2, NP_F16)
    cf16[0:D2, W3_OFF:W3_OFF + D3] = np.asarray(W3, NP_F16)
    cf16[0:D3, W4_OFF:W4_OFF + 1] = np.asarray(W4, NP_F16)
    bia = np.zeros((D1, 3), dtype=np.float32)
    bia[0:D1, 0] = np.asarray(b1, np.float32)
    bia[0:D2, 1] = np.asarray(b2, np.float32)
    bia[0:D3, 2] = np.asarray(b3, np.float32)
    base = {
        "cf16": cf16,
        "w18": w18,
        "bia": bia,
    }
    xs = x.reshape(B, PIX, C)
    maps = []
    for c in range(NCORES):
        xc = xs[c * BPC:(c + 1) * BPC].reshape(NPIX, C)
        # natural GAP copies: tiles 0-11 at [128, t*C:(t+1)*C), tile 12
        # separate [32, C]
        xg = np.ascontiguousarray(
            xc[:1536].reshape(12, P, C).transpose(1, 0, 2).reshape(P, 12 * C)
        ).astype(NP_F16)
        xg3 = np.ascontiguousarray(xc[1536:1568]).astype(NP_F16)
        # transposed chain copy: per-super contiguous [128, NCH*s_sz]
        xct3 = xc.T.reshape(NCH, P, NPIX).transpose(1, 0, 2)
        blocks = []
        for si, tlist in enumerate(SUPER):
            s_off = TILE_OFF[tlist[0]]
            blocks.append(
                xct3[:, :, s_off:s_off + SUP_SZ[si]].reshape(P, -1))
        xt = np.ascontiguousarray(
            np.concatenate(blocks, axis=1)).astype(NP_FP8)
        maps.append({"xg": xg, "xg3": xg3, "xt": xt, **base})
    return maps


def kernel(x, W1, b1, W2, b2, W3, b3, W4, b4, _profile=False, **_ignored):
    nc = build_program(float(np.asarray(b4, np.float32).reshape(-1)[0]))
    in_maps = make_in_maps(x, W1, b1, W2, b2, W3, b3, W4, b4)
    res = run_bass_kernel_spmd(nc, in_maps, core_ids=list(range(NCORES)),
                               trace=_profile)
    out = np.concatenate([res.results[c]["out"] for c in range(NCORES)], axis=0)
    out = np.ascontiguousarray(out.astype(np.float32))
    if _profile:
        return out, res
    return out
